# revision 35
# baseline (speedup 1.0000x reference)
"""Trainium2 Bass kernel for nn_DisLoss (prototype EMA + masked pairwise exp-sim loss).

Strategy (8 NeuronCores, SPMD, symmetric pair coverage):
  - The sequential per-sample EMA scan factors into independent per-class chains
    (order only matters within a class).  Chains are computed vectorized: lanes =
    distinct labels (sorted by chain length desc), rounds = occurrence index.
    Chain inputs are pre-gathered BY THE HOST into one lane-major array, so the
    device does a single contiguous DMA instead of a slow dma_gather.
  - exp(G)ij = exp(G)ji, so each unordered pair is computed ONCE: core r owns
    row-slab r (1024 classes) and computes blocks against col-slabs r..r+3 at
    full weight plus slab r+4 at half weight (bias=ln(1/2); slab pairs {r,r+4}
    are computed by both endpoint cores).  Within the own slab only the upper
    block triangle is computed.  Row sums come from ACT Exp accum_out; column
    sums are accumulated on the DVE in fp16 and partition-reduced by a
    ones-vector matmul.  Per-class partial sums are scattered (host-provided
    rotation indices) into a global-class-layout DRAM buffer; a ReduceScatter
    over the 8 cores hands every core its own slab's complete sums; ln + mean
    finish on-device and the host sums 8 scalars.
  - Updated prototype rows are cast to fp16 and scatter-added (one
    CounterMachine dma_scatter_add onto host-zeroed rows) into a DRAM proto
    copy; protoT [256, 5120-local] is produced by xbar DMA transposes on the
    Sync HWDGE ring.  Single-occurrence lanes are sorted by local class so
    classes >= 5120 (columns no block on this core reads) become a skippable
    -1 tail of the scatter index list.
"""

import math
from contextlib import ExitStack

import numpy as np

import types as _pytypes

import bass_rust as _bass_rust
import concourse.bass as bass
import concourse.mybir as mybir
import concourse.tile as tile
from concourse import bacc
from concourse.bass_utils import run_bass_kernel_spmd
from concourse.hw_specs import get_activation_tables
from concourse.masks import make_identity
from concourse.tile_rust import add_dep_helper

ACT_SET = "natural_log_exp_and_others"  # contains every ACT func we use


def _pin_act_tables(nc):
    """Force all activations onto one table set: the default chooser alternates
    between exp_and_others and natural_log_exp_and_others, paying ~1.3us per
    reload.  Emptying the other sets' membership (indices preserved) pins it."""

    def patched(self):
        has_act = any(
            isinstance(i, mybir.InstActivation)
            for b in self.main_func.blocks
            for i in b.instructions
        )
        if not has_act:
            return
        tables = [
            (name, fns if name == ACT_SET else type(fns)())
            for name, fns in get_activation_tables(self.m.arch).items()
        ]
        _bass_rust.insert_act_table_loads(self, tables)
        # table-load waits are added AFTER compile()'s last event-semaphore
        # split; re-split so no instruction carries >1 wait (ISA cap)
        _bass_rust.generate_event_semaphores(self)

    nc.insert_act_table_loads = _pytypes.MethodType(patched, nc)

P = 128
C = 8192
D = 256
B = 1024
NCORES = 8
CPC = C // NCORES          # classes per core (1024)
NB = CPC // P              # own row blocks (8)
NS = 5                     # col slabs each core touches (own + 3 full + 1 half)
CU = NS * CPC              # used local columns (5120)
TEMP = 0.1
BASE_TEMP = 0.1
LN_HALF = math.log(0.5)

F32 = mybir.dt.float32
F16 = mybir.dt.float16  # logits matmul operand dtype (loss rel err ~1e-7 vs fp32)
I16 = mybir.dt.int16


def _ins(x):
    return getattr(x, "ins", x)


def _chain_structure(labels):
    """Group sample indices by class; lanes sorted by chain length desc."""
    occ = {}
    for t, c in enumerate(labels):
        occ.setdefault(int(c), []).append(t)
    lanes = sorted(occ.items(), key=lambda kv: (-len(kv[1]), kv[0]))
    S = len(lanes)
    R = len(lanes[0][1])
    S_r = [sum(1 for _, ts in lanes if len(ts) > r) for r in range(R)]
    return lanes, S, R, S_r


def build_program(S, R, S_r, NT, NFT, fo_list, n_scat):
    """One SPMD Bass program; all shape-relevant values are rotation-invariant."""
    nc = bacc.Bacc("TRN2", target_bir_lowering=False, debug=False, num_devices=NCORES)
    _pin_act_tables(nc)
    # ufg = host-pregathered [proto rows of distinct labels; per-round feature
    # rows], lane-major: ufg[p, t, :] is lane p of tile t (per-core lane order).
    ufg_d = nc.declare_dram_parameter("ufg", [P, NT + NFT, D], F32, isOutput=False)
    # proto16 has the to-be-updated rows HOST-ZEROED so dma_scatter_add == assign
    proto16 = nc.declare_dram_parameter("proto16", [C, D], F16, isOutput=False)
    sidx_d = nc.declare_dram_parameter("sidx", [P, NT * P // 16], I16, isOutput=False)
    # bounce-combine scatter targets: cols 0..2 = 40 colp rows, col 3 = 8 row rows
    scidx_d = nc.declare_dram_parameter("scidx", [P, 4], I16, isOutput=False)
    # colsel[:, k*128+m] = (m == k): ones-matmul with colsel chunk k folds the
    # partition-sum of acc cols [k*128,(k+1)*128) onto PSUM row k
    colsel_d = nc.declare_dram_parameter("colsel", [P, (CU // P) * P], F16, isOutput=False)
    out_d = nc.declare_dram_parameter("partial", [1, 1], F32, isOutput=True)

    with tile.TileContext(nc) as tc:
        with ExitStack() as ctx:
            aux = ctx.enter_context(tc.tile_pool(name="aux", bufs=1))
            chainp = ctx.enter_context(tc.tile_pool(name="chain", bufs=1))
            psp = ctx.enter_context(tc.tile_pool(name="ps", bufs=2, space="PSUM"))
            bigp = ctx.enter_context(tc.tile_pool(name="big", bufs=1))
            # one scr slot per main-loop group: a reused slot would make the
            # next exp wait on BOTH PE and the DVE reader of the old contents,
            # tripping TRN2's one-wait-per-instruction limit (walrus rejects)
            scrp = ctx.enter_context(tc.tile_pool(name="scr", bufs=3 * NB))
            dram = ctx.enter_context(tc.tile_pool(name="dram", bufs=1, space="DRAM"))

            # chain-input load first: everything in the front half waits on it
            ufg = chainp.tile([P, NT + NFT, D], F32)
            nc.sync.dma_start(ufg[:, :, :], ufg_d[:, :, :])
            sidx_sb = aux.tile([P, NT * P // 16], I16)
            nc.sync.dma_start(sidx_sb[:], sidx_d[:])
            scidx_sb = aux.tile([P, 4], I16)
            nc.sync.dma_start(scidx_sb[:], scidx_d[:])
            colsel_sb = aux.tile([P, (CU // P) * P], F16)
            nc.sync.dma_start(colsel_sb[:], colsel_d[:])

            ident = aux.tile([P, P], F32)
            make_identity(nc, ident[:])
            # fp16 identity + (-BIG)*identity: one extra PE matmul per row block
            # adds -60000 to the diagonal logit before exp -> exp(10*-60000) == 0
            id16 = aux.tile([P, P], F16)
            nc.vector.tensor_copy(id16[:], ident[:])
            negid16 = aux.tile([P, P], F16)
            nc.vector.tensor_scalar_mul(negid16[:], id16[:], -60000.0)
            ones_sb = aux.tile([P, 1], F32)
            nc.vector.memset(ones_sb[:], 1.0)
            lnhalf = aux.tile([P, 1], F32)
            nc.vector.memset(lnhalf[:], LN_HALF)
            zeros64 = aux.tile([64, P], F32)
            nc.vector.memset(zeros64[:], 0.0)
            # force the (single) activation table set to load while DMAs run
            dummy = aux.tile([1, 1], F32)
            nc.scalar.activation(
                out=dummy[:], in_=dummy[0:1, 0:1], func=mybir.ActivationFunctionType.Ln
            )
            # consume the ufg-DMA wait on the ACT engine here: a later chain op
            # needing both this and a DVE wait would exceed the 1-wait ISA cap
            # (bacc's event-semaphore splitter misses this corner)
            dummy2 = aux.tile([1, 1], F32)
            nc.scalar.activation(
                out=dummy2[:], in_=ufg[0:1, 0, 0:1], func=mybir.ActivationFunctionType.Ln
            )

            # ---- chain compute (replicated values, per-core lane order) ----
            u = ufg[:, 0:NT, :]
            fg = ufg[:, NT : NT + NFT, :]
            sqd = chainp.tile([P, D], F32)
            dvet = chainp.tile([1, NT + NFT], F32)
            # norm scratch indexed by a globally-unique column per use (rounds
            # use NT + fo_r + t... wait, finalize uses 0..NT, rounds fo_r+t >= NT)
            n2 = chainp.tile([P, NT + NFT], F32)
            lnb = chainp.tile([P, NT + NFT], F32)
            rinv = chainp.tile([P, NT + NFT], F32)

            # Deferred normalization: track v_{k+1} = v_k + ||v_k|| * f_k (same
            # direction as normalize-each-step since normalize is scale-invariant),
            # then normalize once at the end.  Round 0 has ||v_0|| = 1 exactly.
            fscl = chainp.tile([P, D], F32)
            for r in range(R):
                Sr = S_r[r]
                ntf = Sr // P
                rem = Sr % P
                fo = fo_list[r]
                ntr = ntf + (1 if rem else 0)
                if r == 0:
                    if ntf:
                        nc.vector.tensor_add(
                            u[:, 0:ntf, :], u[:, 0:ntf, :], fg[:, fo : fo + ntf, :]
                        )
                    if rem:
                        nc.vector.tensor_add(
                            u[0:rem, ntf, :], u[0:rem, ntf, :], fg[0:rem, fo + ntf, :]
                        )
                    continue
                for t in range(ntr):
                    pp = P if t < ntf else rem
                    k = fo + t  # globally-unique scratch column for this use
                    # ||v||^2 via ACT Square + free-axis accumulate
                    nc.scalar.activation(
                        out=sqd[0:pp, :],
                        in_=u[0:pp, t, :],
                        func=mybir.ActivationFunctionType.Square,
                        accum_out=n2[0:pp, k : k + 1],
                    )
                    # ||v|| = exp(0.5*ln(n2)); the Sqrt table is low-precision
                    # (65536-ULP budget) while Ln/Exp are ~2 ULP and share a set
                    nc.scalar.activation(
                        out=lnb[0:pp, k : k + 1],
                        in_=n2[0:pp, k : k + 1],
                        func=mybir.ActivationFunctionType.Ln,
                    )
                    nc.scalar.activation(
                        out=rinv[0:pp, k : k + 1],
                        in_=lnb[0:pp, k : k + 1],
                        func=mybir.ActivationFunctionType.Exp,
                        scale=0.5,
                    )
                    # absorb the ACT->DVE wait on a trivial op so the mul below
                    # carries a single wait (one-wait-per-instruction limit)
                    nc.vector.tensor_copy(dvet[:, k : k + 1], rinv[0:1, k : k + 1])
                    nc.vector.tensor_scalar_mul(
                        fscl[0:pp, :], fg[0:pp, fo + t, :], rinv[0:pp, k : k + 1]
                    )
                    nc.vector.tensor_add(u[0:pp, t, :], u[0:pp, t, :], fscl[0:pp, :])

            # ---- normalize+cast fused ----
            tb = 0 if R == 1 else (S_r[1] + P - 1) // P
            u16 = chainp.tile([P, NT, D], F16)

            def finalize(lo, hi):
                if hi <= lo:
                    return
                for t in range(lo, hi):
                    nc.scalar.activation(
                        out=sqd[:],
                        in_=u[:, t, :],
                        func=mybir.ActivationFunctionType.Square,
                        accum_out=n2[:, t : t + 1],
                    )
                nc.scalar.activation(
                    out=lnb[:, lo:hi], in_=n2[:, lo:hi], func=mybir.ActivationFunctionType.Ln
                )
                nc.scalar.activation(
                    out=rinv[:, lo:hi],
                    in_=lnb[:, lo:hi],
                    func=mybir.ActivationFunctionType.Exp,
                    scale=-0.5,
                )
                nc.vector.tensor_copy(dvet[:, lo : lo + 1], rinv[0:1, lo : lo + 1])
                for t in range(lo, hi):
                    nc.vector.tensor_scalar_mul(
                        u16[:, t, :], u[:, t, :], rinv[:, t : t + 1]
                    )

            finalize(tb, NT)  # overlaps rounds >=1 (scheduler is dep-driven)
            finalize(0, tb)

            # GPSIMD touch ops: pull each cross-engine wait onto its own
            # trivial instruction (one-wait-per-instruction ISA limit)
            gpt_i = chainp.tile([1, 1], I16)
            gpt_h = chainp.tile([1, NT, 1], F16)
            nc.gpsimd.tensor_copy(gpt_i[:], sidx_sb[0:1, 0:1])
            nc.gpsimd.tensor_copy(gpt_h[:], u16[0:1, 0:NT, 0:1])
            # one merged scatter-add of the n_scat real rows (trailing -1 skipped)
            scat = nc.gpsimd.dma_scatter_add(
                out_ap=proto16[:, :],
                in_ap=u16[:, 0:NT, :],
                idxs_ap=sidx_sb[:, :],
                num_idxs=NT * P,
                num_idxs_reg=n_scat,
                elem_size=D,
                single_packet=False,
            )

            # ---- protoT (local cols 0..CU) via xbar DMA transpose on Sync ----
            ptT = [bigp.tile([P, CU], F16, name=f"ptT{h}", tag=f"ptT{h}") for h in range(2)]
            RC = 1280  # 4 chunks
            for rc in range(CU // RC):
                for h in range(2):
                    tr = nc.sync.dma_start_transpose(
                        ptT[h][:, rc * RC : (rc + 1) * RC],
                        proto16[rc * RC : (rc + 1) * RC, h * P : (h + 1) * P],
                    )
                    add_dep_helper(
                        _ins(tr), _ins(scat), sync=True, reason="transpose after scatter"
                    )

            # ---- symmetric main loop ----
            # per own block-row b: W0/W1 cover [b*128, 4096) at weight 1 (W0 holds
            # the diagonal block first), W2 = [4096, 5120) at weight 1/2.
            acc = bigp.tile([P, CU], F16)   # fp16 column-partial accumulator
            rs = bigp.tile([P, NB, 3], F32)
            for b in range(NB):
                wins = [
                    (b * P, b * P + 2048, 0.0),
                    (b * P + 2048, 4096, 0.0),
                    (4096, CU, lnhalf[:]),
                ]
                for gi, (lo, hi, bias) in enumerate(wins):
                    width = hi - lo
                    ps = psp.tile([P, 2048], F32, tag="ps")
                    for h in range(2):
                        nstr = (width + 511) // 512
                        for s in range(nstr):
                            c0 = lo + s * 512
                            c1 = min(c0 + 512, hi)
                            nc.tensor.matmul(
                                out=ps[:, s * 512 : s * 512 + (c1 - c0)],
                                lhsT=ptT[h][:, b * P : (b + 1) * P],
                                rhs=ptT[h][:, c0:c1],
                                start=(h == 0),
                                stop=(h == 1) and not (gi == 0 and s == 0),
                            )
                    if gi == 0:
                        # diagonal logit -> -inf (own block's diag sits at offset 0)
                        nc.tensor.matmul(
                            out=ps[:, 0:P],
                            lhsT=negid16[:],
                            rhs=id16[:],
                            start=False,
                            stop=True,
                        )
                    scr = scrp.tile([P, 2048], F16, tag="esc")
                    nc.scalar.activation(
                        out=scr[:, 0:width],
                        in_=ps[:, 0:width],
                        func=mybir.ActivationFunctionType.Exp,
                        scale=1.0 / TEMP,
                        bias=bias,
                        accum_out=rs[:, b, gi : gi + 1],
                    )
                    # column partials (fp16 DVE): b==0 initializes the window
                    if b == 0:
                        nc.vector.tensor_copy(acc[:, lo:hi], scr[:, 0:width])
                    else:
                        nc.vector.tensor_add(
                            acc[:, lo:hi], acc[:, lo:hi], scr[:, 0:width]
                        )
                    if gi == 0:
                        # diag block must not contribute column partials (its
                        # pairs are fully covered by this block's row sums)
                        nc.vector.tensor_tensor(
                            out=acc[:, b * P : (b + 1) * P],
                            in0=acc[:, b * P : (b + 1) * P],
                            in1=scr[:, 0:P],
                            op=mybir.AluOpType.subtract,
                        )

            # ---- combine partials into global-class layout, ReduceScatter ----
            rs_in = dram.tile([64, P], F32)
            rs_out = dram.tile([NB, P], F32)
            z = nc.sync.dma_start(rs_in[:, :], zeros64[:, :])

            # row sums: reduce the 3 groups, transpose to [8, 128] (class-major)
            rowsum = aux.tile([P, NB], F32)
            nc.vector.tensor_reduce(
                out=rowsum[:, :],
                in_=rs[:, :, :],
                axis=mybir.AxisListType.X,
                op=mybir.AluOpType.add,
            )
            rp_ps = psp.tile([P, P], F32, tag="ps")
            nc.tensor.transpose(rp_ps[0:NB, :], rowsum[:, :], ident[:])
            rp_pad = aux.tile([P, 1, P], F32)
            nc.vector.tensor_copy(rp_pad[0:NB, 0, :], rp_ps[0:NB, :])

            # column partials: one-hot-column matmuls fold the partition-sum
            # of acc chunk k onto PSUM row k (rows != k accumulate zeros), so a
            # single partition-aligned DVE copy yields the scatter source
            cps = psp.tile([P, P], F32, tag="ps")
            for k in range(CU // P):
                nc.tensor.matmul(
                    out=cps[:, :],
                    lhsT=colsel_sb[:, k * P : (k + 1) * P],
                    rhs=acc[:, k * P : (k + 1) * P],
                    start=(k == 0),
                    stop=(k == CU // P - 1),
                )
            comb = aux.tile([P, 1, P], F32)
            nc.vector.tensor_copy(comb[0 : CU // P, 0, :], cps[0 : CU // P, :])

            gpt_f = chainp.tile([1, 1], F32)
            gpt_f2 = chainp.tile([1, 1], F32)
            gpt_f3 = chainp.tile([1, 1], F32)
            gpt_i2 = chainp.tile([1, 1], I16)
            gpt_z = chainp.tile([1, 1], F32)
            tz = nc.gpsimd.memset(gpt_z[:], 0.0)
            add_dep_helper(_ins(tz), _ins(z), sync=True, reason="scatter after zero")
            nc.gpsimd.tensor_copy(gpt_i2[:], scidx_sb[0:1, 0:1])
            nc.gpsimd.tensor_copy(gpt_f2[:], comb[0:1, 0, 0:1])
            sc_a = nc.gpsimd.dma_scatter_add(
                out_ap=rs_in[:, :],
                in_ap=comb[:, :, :],
                idxs_ap=scidx_sb[:, 0:3],
                num_idxs=CU // P,
                num_idxs_reg=CU // P,
                elem_size=P,
                single_packet=False,
            )
            nc.gpsimd.tensor_copy(gpt_f3[:], rp_pad[0:1, 0, 0:1])
            sc_b = nc.gpsimd.dma_scatter_add(
                out_ap=rs_in[:, :],
                in_ap=rp_pad[:, :, :],
                idxs_ap=scidx_sb[:, 3:4],
                num_idxs=NB,
                num_idxs_reg=NB,
                elem_size=P,
                single_packet=False,
            )
            add_dep_helper(_ins(sc_b), _ins(sc_a), sync=True, reason="serialize adds")

            cc = nc.gpsimd.collective_compute(
                "ReduceScatter",
                mybir.AluOpType.add,
                replica_groups=[list(range(NCORES))],
                ins=[rs_in.opt()],
                outs=[rs_out.opt()],
            )
            add_dep_helper(_ins(cc), _ins(sc_b), sync=True, reason="rs after combine")

            # ---- own slab: ln(neg_sum/(C-1)), sum, write the partial ----
            osb = aux.tile([NB, P], F32)
            ld = nc.sync.dma_start(osb[:, :], rs_out[:, :])
            add_dep_helper(_ins(ld), _ins(cc), sync=True, reason="load after rs")
            lnv = aux.tile([NB, P], F32)
            lnacc = aux.tile([NB, 1], F32)
            nc.scalar.activation(
                out=lnv[:, :],
                in_=osb[:, :],
                func=mybir.ActivationFunctionType.Ln,
                scale=1.0 / (C - 1),
                accum_out=lnacc[:, :],
            )
            pfin = psp.tile([1, 1], F32, tag="ps")
            nc.tensor.matmul(
                out=pfin[:], lhsT=lnacc[:, :], rhs=ones_sb[0:NB, :], start=True, stop=True
            )
            fsb = aux.tile([1, 1], F32)
            nc.vector.tensor_copy(fsb[:], pfin[:])
            nc.sync.dma_start(out_d[:], fsb[:])

    nc.compile()
    return nc


def _host_meta(labels):
    lanes, S, R, S_r = _chain_structure(labels)
    NT = (S + P - 1) // P
    fo_list = []
    off = 0
    for r in range(R):
        fo_list.append(off)
        off += (S_r[r] + P - 1) // P
    NFT = off

    fflat = np.zeros(NFT * P, dtype=np.int64)
    for r in range(R):
        for L in range(S_r[r]):
            fflat[fo_list[r] * P + L] = lanes[L][1][r]
    lane_class = np.array([c for c, _ in lanes], dtype=np.int64)
    return lanes, S, R, S_r, NT, NFT, fo_list, fflat, lane_class


def _wrap_idx16(flat):
    """CounterMachine index layout: flat[i] at [16*rep + i%16, i//16], 8 replicas."""
    n = len(flat)
    assert n % 16 == 0
    blk = flat.reshape(n // 16, 16).T.astype(np.int16)  # [16, n/16]
    return np.tile(blk, (8, 1))  # [128, n/16]


def prepare(features, prototypes, labels):
    """Host-side specialization: build the SPMD program and per-core inputs."""
    features = np.asarray(features, dtype=np.float32)
    prototypes = np.asarray(prototypes, dtype=np.float32)
    labels_np = np.asarray(labels).astype(np.int64)

    colsel = np.zeros((CU // P, P), dtype=np.float16)
    np.fill_diagonal(colsel, 1.0)
    colsel_host = (
        np.broadcast_to(colsel.reshape(1, CU // P, P), (P, CU // P, P))
        .reshape(P, (CU // P) * P)
        .copy()
    )

    lanes, S, R, S_r, NT, NFT, fo_list, fflat, lane_class = _host_meta(labels_np)
    nc = build_program(S, R, S_r, NT, NFT, fo_list, S)

    # Host-pregathered chain inputs, identical for every core
    gflat = np.zeros((NT + NFT) * P, dtype=np.int64)
    gflat[:S] = lane_class
    ufg_rows = np.concatenate([prototypes[gflat[: NT * P]], features[fflat]])
    ufg_host = np.ascontiguousarray(ufg_rows.reshape(NT + NFT, P, D).transpose(1, 0, 2))

    in_maps = []
    for r0 in range(NCORES):
        rot_class = (lane_class - r0 * CPC) % C

        sflat = np.full(NT * P, -1, dtype=np.int64)  # -1 tail = skipped
        sflat[:S] = rot_class

        proto16c = np.roll(prototypes, -r0 * CPC, axis=0).astype(np.float16)
        proto16c[rot_class] = 0  # scatter-ADD targets must start at zero

        # bounce-combine targets: colp row k -> global row (k + 8*r0) % 64;
        # rowsum row j -> global row (j + 8*r0) (own slab)
        cidx = np.full(48, -1, dtype=np.int64)
        cidx[: CU // P] = (np.arange(CU // P) + NB * r0) % (C // P)
        ridx = np.full(16, -1, dtype=np.int64)
        ridx[:NB] = np.arange(NB) + NB * r0
        scidx = np.concatenate([_wrap_idx16(cidx), _wrap_idx16(ridx)], axis=1)

        in_maps.append(
            {
                "ufg": ufg_host,
                "proto16": np.ascontiguousarray(proto16c),
                "sidx": _wrap_idx16(sflat),
                "scidx": scidx,
                "colsel": colsel_host,
            }
        )

    return nc, in_maps


def kernel(features, prototypes, labels):
    nc, in_maps = prepare(features, prototypes, labels)
    res = run_bass_kernel_spmd(nc, in_maps, list(range(NCORES)))
    partials = [float(res.results[i]["partial"][0, 0]) for i in range(NCORES)]
    loss = (TEMP / BASE_TEMP) * (sum(partials) / C)
    return np.asarray(loss, dtype=np.float32)


# revision 36
# speedup vs baseline: 1.2532x; 1.2532x over previous
"""Trainium2 Bass kernel for nn_DisLoss (prototype EMA + masked pairwise exp-sim loss).

Strategy (8 NeuronCores, SPMD):
  - The sequential per-sample EMA scan factors into independent per-class chains
    (order only matters within a class).  Chains are computed vectorized: lanes =
    distinct labels (sorted by chain length desc), rounds = occurrence index.
  - Chain inputs (distinct-label proto rows + per-round feature rows) are
    pre-gathered BY THE HOST into one lane-major array (identical for all
    cores), so the device does a single contiguous DMA instead of a slow
    gpsimd dma_gather descriptor storm.
  - Each core receives class-rotated copies of the prototypes so that "its" 1024
    rows are rows 0..1023; one compiled program serves all 8 cores.
  - Updated rows are cast to fp16 and scattered (indirect DMA, one call per
    2048-row quarter of the proto matrix) into a host-cast fp16 DRAM proto
    copy; protoT [256, 8192] is produced by xbar DMA transposes on the Sync
    HWDGE ring only (keeping the ACT engine free for the exp stream), each
    row-chunk depending only on its own quarter's scatter.  Each core computes
    its [1024, 8192] block of exp(P'P'^T/T) in fp16 matmuls (fp32 PSUM
    accumulate; loss rel err ~1e-7 vs fp32), with the diagonal masked to -BIG
    before the exp, ACT Exp accum_out row-sums, Ln, and an on-chip partial
    reduction.  The host sums 8 scalars.
"""

import math
from contextlib import ExitStack

import numpy as np

import types as _pytypes

import bass_rust as _bass_rust
import concourse.bass as bass
import concourse.mybir as mybir
import concourse.tile as tile
from concourse import bacc
from concourse.bass_utils import run_bass_kernel_spmd
from concourse.hw_specs import get_activation_tables
from concourse.masks import make_identity
from concourse.tile_rust import add_dep_helper

ACT_SET = "natural_log_exp_and_others"  # contains every ACT func we use


def _pin_act_tables(nc):
    """Force all activations onto one table set: the default chooser alternates
    between exp_and_others and natural_log_exp_and_others, paying ~1.3us per
    reload.  Emptying the other sets' membership (indices preserved) pins it."""

    def patched(self):
        has_act = any(
            isinstance(i, mybir.InstActivation)
            for b in self.main_func.blocks
            for i in b.instructions
        )
        if not has_act:
            return
        tables = [
            (name, fns if name == ACT_SET else type(fns)())
            for name, fns in get_activation_tables(self.m.arch).items()
        ]
        _bass_rust.insert_act_table_loads(self, tables)

    nc.insert_act_table_loads = _pytypes.MethodType(patched, nc)

P = 128
C = 8192
D = 256
B = 1024
NCORES = 8
CPC = C // NCORES          # classes per core (1024)
NB = CPC // P              # own row blocks (8)
CT = C // P                # class tiles (64)
TEMP = 0.1
BASE_TEMP = 0.1

F32 = mybir.dt.float32
F16 = mybir.dt.float16  # logits matmul operand dtype (loss rel err ~1e-7 vs fp32)
I32 = mybir.dt.int32
I16 = mybir.dt.int16


def _ins(x):
    return getattr(x, "ins", x)


def _chain_structure(labels):
    """Group sample indices by class; lanes sorted by chain length desc."""
    occ = {}
    for t, c in enumerate(labels):
        occ.setdefault(int(c), []).append(t)
    lanes = sorted(occ.items(), key=lambda kv: (-len(kv[1]), kv[0]))
    S = len(lanes)
    R = len(lanes[0][1])
    S_r = [sum(1 for _, ts in lanes if len(ts) > r) for r in range(R)]
    return lanes, S, R, S_r


def build_program(S, R, S_r, NT, NFT, fo_list):
    """One SPMD Bass program; all shape-relevant values are rotation-invariant."""
    nc = bacc.Bacc("TRN2", target_bir_lowering=False, debug=False, num_devices=NCORES)
    _pin_act_tables(nc)
    # ufg = host-pregathered [proto rows of distinct labels; per-round feature
    # rows], lane-major: ufg[p, t, :] is lane p of tile t.  Rotation-invariant.
    ufg_d = nc.declare_dram_parameter("ufg", [P, NT + NFT, D], F32, isOutput=False)
    # proto16 has the to-be-updated rows HOST-ZEROED so dma_scatter_add == assign
    proto16 = nc.declare_dram_parameter("proto16", [C, D], F16, isOutput=False)
    sidx_d = nc.declare_dram_parameter("sidx", [P, NT * P // 16], I16, isOutput=False)
    out_d = nc.declare_dram_parameter("partial", [1, 1], F32, isOutput=True)

    with tile.TileContext(nc) as tc:
        with ExitStack() as ctx:
            aux = ctx.enter_context(tc.tile_pool(name="aux", bufs=1))
            chainp = ctx.enter_context(tc.tile_pool(name="chain", bufs=1))
            psp = ctx.enter_context(tc.tile_pool(name="ps", bufs=2, space="PSUM"))
            bigp = ctx.enter_context(tc.tile_pool(name="big", bufs=1))
            scrp = ctx.enter_context(tc.tile_pool(name="scr", bufs=2))

            ident = aux.tile([P, P], F32)
            make_identity(nc, ident[:])
            # fp16 identity + (-BIG)*identity: one extra PE matmul per row block
            # adds -60000 to the diagonal logit before exp -> exp(10*-60000) == 0
            id16 = aux.tile([P, P], F16)
            nc.vector.tensor_copy(id16[:], ident[:])
            negid16 = aux.tile([P, P], F16)
            nc.vector.tensor_scalar_mul(negid16[:], id16[:], -60000.0)
            ones_sb = aux.tile([P, 1], F32)
            nc.vector.memset(ones_sb[:], 1.0)
            # force the (single) activation table set to load while DMAs run
            dummy = aux.tile([1, 1], F32)
            nc.scalar.activation(
                out=dummy[:], in_=ones_sb[0:1, 0:1], func=mybir.ActivationFunctionType.Ln
            )

            sidx_sb = aux.tile([P, NT * P // 16], I16)
            nc.sync.dma_start(sidx_sb[:], sidx_d[:])

            # ---- chain compute (replicated) ----
            ufg = chainp.tile([P, NT + NFT, D], F32)
            u = ufg[:, 0:NT, :]
            fg = ufg[:, NT : NT + NFT, :]
            sqd = chainp.tile([P, D], F32)
            n2 = chainp.tile([P, NT], F32)
            lnb = chainp.tile([P, NT], F32)
            rinv = chainp.tile([P, NT], F32)
            nc.vector.memset(n2[:], 1.0)

            # one contiguous load (per-partition 19KB lines) replaces dma_gather
            nc.sync.dma_start(ufg[:, :, :], ufg_d[:, :, :])

            # Deferred normalization: track v_{k+1} = v_k + ||v_k|| * f_k (same
            # direction as normalize-each-step since normalize is scale-invariant),
            # then normalize once at the end.  Round 0 has ||v_0|| = 1 exactly.
            fscl = chainp.tile([P, D], F32)
            for r in range(R):
                Sr = S_r[r]
                ntf = Sr // P
                rem = Sr % P
                fo = fo_list[r]
                ntr = ntf + (1 if rem else 0)
                if r == 0:
                    if ntf:
                        nc.vector.tensor_add(
                            u[:, 0:ntf, :], u[:, 0:ntf, :], fg[:, fo : fo + ntf, :]
                        )
                    if rem:
                        nc.vector.tensor_add(
                            u[0:rem, ntf, :], u[0:rem, ntf, :], fg[0:rem, fo + ntf, :]
                        )
                    continue
                for t in range(ntr):
                    pp = P if t < ntf else rem
                    # ||v||^2 via ACT Square + free-axis accumulate (one op,
                    # keeps the DVE free for the mul/add of the next lane tile)
                    nc.scalar.activation(
                        out=sqd[0:pp, :],
                        in_=u[0:pp, t, :],
                        func=mybir.ActivationFunctionType.Square,
                        accum_out=n2[0:pp, t : t + 1],
                    )
                    # ||v|| = exp(0.5*ln(n2)); the Sqrt table is low-precision
                    # (65536-ULP budget) while Ln/Exp are ~2 ULP and share a set
                    nc.scalar.activation(
                        out=lnb[0:pp, t : t + 1],
                        in_=n2[0:pp, t : t + 1],
                        func=mybir.ActivationFunctionType.Ln,
                    )
                    nc.scalar.activation(
                        out=rinv[0:pp, t : t + 1],
                        in_=lnb[0:pp, t : t + 1],
                        func=mybir.ActivationFunctionType.Exp,
                        scale=0.5,
                    )
                    nc.vector.tensor_scalar_mul(
                        fscl[0:pp, :], fg[0:pp, fo + t, :], rinv[0:pp, t : t + 1]
                    )
                    nc.vector.tensor_add(u[0:pp, t, :], u[0:pp, t, :], fscl[0:pp, :])

            # ---- normalize+cast fused, scatter into the fp16 proto copy ----
            # Lanes touched by rounds >=1 all sit in tiles < tb (length-sorted),
            # so tiles tb.. finalize right after round 0, overlapping the rounds.
            tb = 0 if R == 1 else (S_r[1] + P - 1) // P
            u16 = chainp.tile([P, NT, D], F16)

            def finalize(lo, hi):
                if hi <= lo:
                    return
                for t in range(lo, hi):
                    nc.scalar.activation(
                        out=sqd[:],
                        in_=u[:, t, :],
                        func=mybir.ActivationFunctionType.Square,
                        accum_out=n2[:, t : t + 1],
                    )
                nc.scalar.activation(
                    out=lnb[:, lo:hi], in_=n2[:, lo:hi], func=mybir.ActivationFunctionType.Ln
                )
                nc.scalar.activation(
                    out=rinv[:, lo:hi],
                    in_=lnb[:, lo:hi],
                    func=mybir.ActivationFunctionType.Exp,
                    scale=-0.5,
                )
                for t in range(lo, hi):
                    nc.vector.tensor_scalar_mul(
                        u16[:, t, :], u[:, t, :], rinv[:, t : t + 1]
                    )

            finalize(tb, NT)  # overlaps rounds >=1 (emitted above in program order)

            # scatter-add the (zero-target) early tiles while late tiles finish;
            # the optimized gather/scatter CounterMachine path is ~3x cheaper
            # per row than generic indirect_dma_start.  Trailing -1 indices are
            # skipped; num_idxs_reg must count exactly the real (non-pad) ones.
            scats = []
            if NT > tb:
                scats.append(
                    nc.gpsimd.dma_scatter_add(
                        out_ap=proto16[:, :],
                        in_ap=u16[:, tb:NT, :],
                        idxs_ap=sidx_sb[:, tb * 8 : NT * 8],
                        num_idxs=(NT - tb) * P,
                        num_idxs_reg=S - tb * P,
                        elem_size=D,
                        single_packet=False,
                    )
                )

            finalize(0, tb)
            if tb > 0:
                scats.append(
                    nc.gpsimd.dma_scatter_add(
                        out_ap=proto16[:, :],
                        in_ap=u16[:, 0:tb, :],
                        idxs_ap=sidx_sb[:, 0 : tb * 8],
                        num_idxs=tb * P,
                        num_idxs_reg=tb * P,
                        elem_size=D,
                        single_packet=False,
                    )
                )

            # ---- protoT via xbar DMA transpose (no PE work) ----
            # Sync HWDGE ring only: transposes on nc.scalar would occupy the ACT
            # engine's queue and stall the exp stream.
            ptT = [bigp.tile([P, C], F16, name=f"ptT{h}", tag=f"ptT{h}") for h in range(2)]
            RC = 2048  # row-chunked so the first matmuls can start early
            for rc in range(C // RC):
                for h in range(2):
                    tr = nc.sync.dma_start_transpose(
                        ptT[h][:, rc * RC : (rc + 1) * RC],
                        proto16[rc * RC : (rc + 1) * RC, h * P : (h + 1) * P],
                    )
                    for si in scats:
                        add_dep_helper(
                            _ins(tr),
                            _ins(si),
                            sync=True,
                            reason="transpose after scatter",
                        )

            # ---- own row-block x all-columns matmul + exp row sums ----
            GW = 2048  # psum group width: 4 banks, double-buffered = all 8 banks
            NG = C // GW
            NS = GW // 512
            rs = bigp.tile([P, NB * NG], F32)
            rsum = aux.tile([P, NB], F32)
            mp2 = aux.tile([P, NB], F32)
            # g outer: group g only needs transpose chunk g, so matmuls start
            # as soon as the first chunk lands instead of after all four
            for g in range(NG):
                for b in range(NB):
                    ps = psp.tile([P, GW], F32, tag="ps")
                    for h in range(2):
                        for s in range(NS):
                            nc.tensor.matmul(
                                out=ps[:, s * 512 : (s + 1) * 512],
                                lhsT=ptT[h][:, b * P : (b + 1) * P],
                                rhs=ptT[h][:, g * GW + s * 512 : g * GW + (s + 1) * 512],
                                start=(h == 0),
                                stop=(h == 1) and not (g == 0 and s == b // 4),
                            )
                    if g == 0:
                        # own classes sit at rotated cols 0..CPC; row p of block b is
                        # class b*P+p -> accumulate -60000 onto the exact diagonal
                        # (PE-only masking; exp(10 * (logit - 60000)) == 0)
                        nc.tensor.matmul(
                            out=ps[:, b * P : (b + 1) * P],
                            lhsT=negid16[:],
                            rhs=id16[:],
                            start=False,
                            stop=True,
                        )
                    scr = scrp.tile([P, GW], F32, tag="esc")
                    nc.scalar.activation(
                        out=scr[:],
                        in_=ps[:],
                        func=mybir.ActivationFunctionType.Exp,
                        scale=1.0 / TEMP,
                        accum_out=rs[:, b * NG + g : b * NG + g + 1],
                    )
                    if g == NG - 1:
                        # block b is complete: row sums + log overlap the
                        # remaining blocks' matmuls
                        nc.vector.tensor_reduce(
                            out=rsum[:, b : b + 1],
                            in_=rs[:, b * NG : (b + 1) * NG],
                            axis=mybir.AxisListType.X,
                            op=mybir.AluOpType.add,
                        )
                        nc.scalar.activation(
                            out=mp2[:, b : b + 1],
                            in_=rsum[:, b : b + 1],
                            func=mybir.ActivationFunctionType.Ln,
                            scale=1.0 / (C - 1),
                        )
            rp = aux.tile([P, 1], F32)
            nc.vector.tensor_reduce(
                out=rp[:], in_=mp2[:], axis=mybir.AxisListType.X, op=mybir.AluOpType.add
            )
            pfin = psp.tile([1, 1], F32, tag="ps")
            nc.tensor.matmul(out=pfin[:], lhsT=rp[:], rhs=ones_sb[:], start=True, stop=True)
            osb = aux.tile([1, 1], F32)
            nc.vector.tensor_copy(osb[:], pfin[:])
            nc.sync.dma_start(out_d[:], osb[:])

    nc.compile()
    return nc


def _host_meta(labels):
    lanes, S, R, S_r = _chain_structure(labels)
    NT = (S + P - 1) // P
    fo_list = []
    off = 0
    for r in range(R):
        fo_list.append(off)
        off += (S_r[r] + P - 1) // P
    NFT = off

    fflat = np.zeros(NFT * P, dtype=np.int64)
    for r in range(R):
        for L in range(S_r[r]):
            fflat[fo_list[r] * P + L] = lanes[L][1][r]
    lane_class = np.array([c for c, _ in lanes], dtype=np.int64)
    return lanes, S, R, S_r, NT, NFT, fo_list, fflat, lane_class


def _wrap_idx16(flat):
    """CounterMachine index layout: flat[i] at [16*rep + i%16, i//16], 8 replicas."""
    n = len(flat)
    assert n % 16 == 0
    blk = flat.reshape(n // 16, 16).T.astype(np.int16)  # [16, n/16]
    return np.tile(blk, (8, 1))  # [128, n/16]


def prepare(features, prototypes, labels):
    """Host-side specialization: build the SPMD program and per-core inputs."""
    features = np.asarray(features, dtype=np.float32)
    prototypes = np.asarray(prototypes, dtype=np.float32)
    labels_np = np.asarray(labels).astype(np.int64)

    lanes, S, R, S_r, NT, NFT, fo_list, fflat, lane_class = _host_meta(labels_np)
    nc = build_program(S, R, S_r, NT, NFT, fo_list)

    # Host-pregathered chain inputs, identical for every core: tile t<NT lane p
    # holds the proto row of distinct label L=t*128+p (pad: proto row 0); tiles
    # >=NT hold per-round feature rows in lane order (pad: feature row 0).
    gflat = np.zeros((NT + NFT) * P, dtype=np.int64)
    gflat[:S] = lane_class
    ufg_rows = np.concatenate(
        [prototypes[gflat[: NT * P]], features[fflat]]
    )  # [(NT+NFT)*128, 256]
    ufg_host = np.ascontiguousarray(
        ufg_rows.reshape(NT + NFT, P, D).transpose(1, 0, 2)
    )  # [P, NT+NFT, D], lane-major

    proto16_full = prototypes.astype(np.float16)
    proto16_full[lane_class] = 0  # scatter-ADD targets must start at zero

    in_maps = []
    for r0 in range(NCORES):
        rot_class = (lane_class - r0 * CPC) % C  # per-core rotated class ids
        sflat = np.full(NT * P, -1, dtype=np.int64)  # -1 = skipped
        sflat[:S] = rot_class
        protoc16 = np.ascontiguousarray(np.roll(proto16_full, -r0 * CPC, axis=0))
        in_maps.append(
            {
                "ufg": ufg_host,
                "proto16": protoc16,
                "sidx": _wrap_idx16(sflat),
            }
        )

    return nc, in_maps


def kernel(features, prototypes, labels):
    nc, in_maps = prepare(features, prototypes, labels)
    res = run_bass_kernel_spmd(nc, in_maps, list(range(NCORES)))
    partials = [float(res.results[i]["partial"][0, 0]) for i in range(NCORES)]
    loss = (TEMP / BASE_TEMP) * (sum(partials) / C)
    return np.asarray(loss, dtype=np.float32)

